# revision 26
# baseline (speedup 1.0000x reference)
"""GQA attention kernel for Trainium2, 8 NeuronCores.

Problem: B=2, T=2048, E=2048, 32 query heads, 8 KV heads, head_dim=64, causal.
Sharding: 2 (batch) x 4 (tensor-parallel) grid. Each TP rank owns 2 KV heads
(=> 8 query heads, 512 q-channels). Wq/Wkv column-sharded, Wo row-sharded;
per-rank partial outputs are summed on host (cheap vs. on-device collective).

Device kernel per core (all matmuls bf16, fp32 PSUM accumulation):
  1. QT  = Wq_loc  @ x^T   [512, T]   (q channels on partitions, packed so that
                                       q-head i sits at partition half i//4 --
                                       matching its KV head's partition half)
  2. KT  = Wk_loc  @ x^T   [128, T]
  3. V   = x @ Wv_loc^T    [T, 128]   (natural layout, keys on partitions)
  4. flash-style causal attention without max-subtraction (scores ~ N(0,1)):
       S^T[k,q] = K^T.T @ Q^T, P^T = exp(S^T/8), accumulated
       out^T[hd,q] = sum_k (V_aug.T @ P^T) with a ones column giving row sums.
     Normalization is deferred: unnormalized out^T and the row sums are
     staged to SBUF; per 512-query chunk one batched reciprocal + a
     selection-matrix matmul broadcasts 1/sum across partitions, then a
     single tensor_mul normalizes all 128 channel rows at once.
  5. out_partial = att^T.T @ Wo_loc^T  [T, E] fp32, interleaved per q-chunk
"""

import numpy as np
import ml_dtypes

import concourse.bass as bass
import concourse.mybir as mybir
import concourse.tile as tile
from concourse import bacc
from concourse.bass_utils import run_bass_kernel_spmd

E = 2048
T = 2048
HQ = 32
HKV = 8
HD = 64
G = 4            # query heads per kv head
P = 128
QL = 512         # local q channels per rank (8 heads)
KVL = 128        # local k (or v) channels per rank (2 heads)
NB = 2           # batches
NR = 4           # tensor-parallel ranks
SCALE = 1.0 / 8.0

BF16 = mybir.dt.bfloat16
F32 = mybir.dt.float32

_CACHE = {}


def _build_program():
    from contextlib import ExitStack

    nc = bacc.Bacc(None, target_bir_lowering=False, debug=False)
    xT = nc.declare_dram_parameter("xT", [E, T], BF16, isOutput=False)
    wqT = nc.declare_dram_parameter("wqT", [E, QL], BF16, isOutput=False)
    wkvT = nc.declare_dram_parameter("wkvT", [E, 2 * KVL], BF16, isOutput=False)
    woT = nc.declare_dram_parameter("woT", [QL, E], BF16, isOutput=False)
    tri = nc.declare_dram_parameter("tri", [P, P], BF16, isOutput=False)
    sel = nc.declare_dram_parameter("sel", [8, G * P], BF16, isOutput=False)
    out = nc.declare_dram_parameter("out", [T, E], F32, isOutput=True)

    EC = E // P      # 16 contraction chunks
    TC = T // P      # 16 t chunks of 128
    T4 = T // 512    # 4 t chunks of 512

    with tile.TileContext(nc) as tc, ExitStack() as ctx:
        const = ctx.enter_context(tc.tile_pool(name="const", bufs=1))
        work = ctx.enter_context(tc.tile_pool(name="work", bufs=4))
        tpool = ctx.enter_context(tc.tile_pool(name="tpool", bufs=2))
        outp = ctx.enter_context(tc.tile_pool(name="outp", bufs=4))
        mm = ctx.enter_context(tc.tile_pool(name="mm", bufs=2, space="PSUM"))
        flp = ctx.enter_context(tc.tile_pool(name="flp", bufs=2, space="PSUM"))
        otp = ctx.enter_context(tc.tile_pool(name="otp", bufs=2, space="PSUM"))

        # ---- persistent SBUF tensors ----
        xts = const.tile([P, EC, T], BF16, tag="xts")          # x^T
        wq_s = const.tile([P, EC, QL], BF16, tag="wq")         # Wq^T (packed col order)
        wkv_s = const.tile([P, EC, 2 * KVL], BF16, tag="wkv")  # [K | V] cols
        wo_s = const.tile([P, QL // P, E], BF16, tag="wo")     # Wo^T (packed row order)
        tri_s = const.tile([P, P], BF16, tag="tri")            # causal mask: tri[k,q]=1 iff q>=k
        sel_s = const.tile([8, G, P], BF16, tag="sel")         # broadcast selectors
        qt_s = const.tile([P, G, T], BF16, tag="qt")           # Q^T
        kt_s = const.tile([P, T], BF16, tag="kt")              # K^T
        vag_s = const.tile([P, TC, 2, 66], BF16, tag="vag")    # V_aug per (tchunk, kvhead)
        at_s = const.tile([P, G, T], BF16, tag="at")           # att out^T (normalized in place)
        stage_s = const.tile([P, 8, 512], F32, tag="stage")    # sums staging on partition 64
        sums_s = const.tile([8, T4, 512], F32, tag="sums")     # row sums per (i, qc)
        rec_s = const.tile([8, T4, 512], F32, tag="rec")
        recb_s = const.tile([8, T4, 512], BF16, tag="recb")

        # x^T is loaded per 512-t chunk so the first projections start early;
        # the weights needed by the first psum groups come first
        xTr = xT.rearrange("(o p) t -> p o t", p=P)
        nc.sync.dma_start(out=xts[:, :, 0:512], in_=xTr[:, :, 0:512])
        nc.sync.dma_start(out=wkv_s, in_=wkvT.rearrange("(o p) c -> p o c", p=P))
        nc.sync.dma_start(out=wq_s, in_=wqT.rearrange("(o p) q -> p o q", p=P))
        for t4 in range(1, T4):
            nc.sync.dma_start(
                out=xts[:, :, t4 * 512:(t4 + 1) * 512],
                in_=xTr[:, :, t4 * 512:(t4 + 1) * 512],
            )
        nc.sync.dma_start(out=wo_s, in_=woT.rearrange("(o p) e -> p o e", p=P))
        nc.sync.dma_start(out=tri_s, in_=tri[:])
        nc.sync.dma_start(out=sel_s, in_=sel.rearrange("p (b m) -> p b m", b=G))
        nc.vector.memset(vag_s[:, :, :, 64:66], 1.0)  # ones col (65) + pad (66)

        # ---- projection psum groups (emitted directly or as filler thunks) --
        def emit_qproj(g, t4):
            ps = flp.tile([P, 512], F32, tag="fl", name="ps")
            for e in range(EC):
                nc.tensor.matmul(
                    ps,
                    lhsT=wq_s[:, e, g * P:(g + 1) * P],
                    rhs=xts[:, e, t4 * 512:(t4 + 1) * 512],
                    start=(e == 0),
                    stop=(e == EC - 1),
                )
            nc.vector.tensor_copy(out=qt_s[:, g, t4 * 512:(t4 + 1) * 512], in_=ps)

        def emit_kproj(t4):
            ps = flp.tile([P, 512], F32, tag="fl", name="ps")
            for e in range(EC):
                nc.tensor.matmul(
                    ps,
                    lhsT=wkv_s[:, e, 0:KVL],
                    rhs=xts[:, e, t4 * 512:(t4 + 1) * 512],
                    start=(e == 0),
                    stop=(e == EC - 1),
                )
            nc.vector.tensor_copy(out=kt_s[:, t4 * 512:(t4 + 1) * 512], in_=ps)

        def emit_vproj(t):
            ps = flp.tile([P, 512], F32, tag="fl", name="ps")
            psv = ps[:, :KVL]
            for e in range(EC):
                nc.tensor.matmul(
                    psv,
                    lhsT=xts[:, e, t * P:(t + 1) * P],
                    rhs=wkv_s[:, e, KVL:2 * KVL],
                    start=(e == 0),
                    stop=(e == EC - 1),
                )
            for h in range(2):
                nc.vector.tensor_copy(
                    out=vag_s[:, t, h, 0:HD], in_=psv[:, h * HD:(h + 1) * HD]
                )

        # ---- attention + per-chunk normalization + O projection ----
        # local q-head i (0..7): kv half = i//4, qt chunk = i%4, partition base = (i//4)*64
        # Heads are processed in (half0, half1) pairs so one head's exp (ACT)
        # overlaps the other's matmuls (PE); normalization + O projection of
        # chunk qc is emitted after the attention of chunk qc+1 (software
        # pipelining) so the reciprocal never stalls the PE queue.
        from collections import deque

        fill_q = deque()   # small PE work items with no ACT dependency

        def emit_attention_pair(qc, g, copies_on_scalar=False):
            # head pair (i=g, i=g+4): both halves' scores go into one 2-bank
            # psum tile so a single ACT instruction exps both; S matmuls run
            # two k-blocks ahead of the AV matmuls so the PE never waits on
            # the activation (stays at full clock). When the stagger runs out
            # of S matmuls (pair tail), filler thunks keep the PE stream fed.
            q_sl = slice(qc * 512, (qc + 1) * 512)
            kmax = 4 * qc + 3
            ots = [otp.tile([65, 512], F32, tag="ot", name=f"ot{h}") for h in range(2)]
            st_tiles = {}

            def c0_of(kb):
                return max(kb - 4 * qc, 0) * P

            def emit_s(kb):
                c0 = c0_of(kb)
                st2 = mm.tile([P, 2, 512], F32, tag="st", name="st2")
                st_tiles[kb] = st2
                for half in range(2):
                    pb = half * HD
                    nc.tensor.matmul(
                        st2[:, half, c0:512],
                        lhsT=kt_s[pb:pb + HD, kb * P:(kb + 1) * P],
                        rhs=qt_s[pb:pb + HD, g, qc * 512 + c0:(qc + 1) * 512],
                        start=True,
                        stop=True,
                    )

            emit_s(0)
            if fill_q:
                fill_q.popleft()()   # covers exp(0) latency at pair start
            for kb in range(kmax + 1):
                if kb + 1 <= kmax:
                    emit_s(kb + 1)
                elif fill_q:
                    fill_q.popleft()()
                j = kb - 4 * qc
                c0 = c0_of(kb)
                st2 = st_tiles.pop(kb)
                pt2 = work.tile([P, 2, 512], BF16, tag="pt", name="pt2")
                nc.scalar.activation(
                    out=pt2[:, :, c0:512],
                    in_=st2[:, :, c0:512],
                    func=mybir.ActivationFunctionType.Exp,
                    scale=SCALE,
                )
                for half in range(2):
                    if j >= 0:
                        nc.vector.tensor_mul(
                            out=pt2[:, half, c0:c0 + P],
                            in0=pt2[:, half, c0:c0 + P],
                            in1=tri_s,
                        )
                    nc.tensor.matmul(
                        ots[half][:, c0:512],
                        lhsT=vag_s[:, kb, half, 0:65],
                        rhs=pt2[:, half, c0:512],
                        start=(kb == 0),
                        stop=(kb == kmax),
                        skip_group_check=True,
                    )

            # stage unnormalized output + row sums; normalize later in bulk
            ceng = nc.scalar.copy if copies_on_scalar else nc.vector.tensor_copy
            for half in range(2):
                ot = ots[half]
                i = 4 * half + g
                ceng(out=stage_s[64:65, i, :], in_=ot[64:65, :])
                if half == 0:
                    ceng(out=at_s[0:HD, g, q_sl], in_=ot[0:HD, :])
                else:
                    # DVE lanes can't cross partitions; write at base 0 then
                    # DMA-shift SBUF->SBUF into partitions 64..127
                    tmp = tpool.tile([HD, 512], BF16, tag="tmp")
                    ceng(out=tmp, in_=ot[0:HD, :])
                    nc.sync.dma_start(out=at_s[HD:P, g, q_sl], in_=tmp)
            for i in (g, 4 + g):
                nc.sync.dma_start(
                    out=sums_s[i:i + 1, qc, :], in_=stage_s[64:65, i, :]
                )

        def emit_norm(qc):
            q_sl = slice(qc * 512, (qc + 1) * 512)
            # batched normalization for this q chunk: 8 rows at once
            nc.vector.reciprocal_approx_fast(out=rec_s[0:8, qc, :], in_=sums_s[0:8, qc, :])
            nc.scalar.copy(out=recb_s[0:8, qc, :], in_=rec_s[0:8, qc, :])
            for g in range(G):
                bc = flp.tile([P, 512], F32, tag="fl", name="bc")
                nc.tensor.matmul(
                    bc,
                    lhsT=sel_s[:, g, :],
                    rhs=recb_s[0:8, qc, :],
                    start=True,
                    stop=True,
                )
                nc.vector.tensor_mul(
                    out=at_s[:, g, q_sl], in0=at_s[:, g, q_sl], in1=bc
                )

        def emit_oproj_eo(t, eo, on_scalar=False):
            # ---- O projection, one [128 t, 512 eo] psum group ----
            ps = flp.tile([P, 512], F32, tag="fl", name="ps")
            for cc in range(QL // P):
                nc.tensor.matmul(
                    ps,
                    lhsT=at_s[:, cc, t * P:(t + 1) * P],
                    rhs=wo_s[:, cc, eo * 512:(eo + 1) * 512],
                    start=(cc == 0),
                    stop=(cc == QL // P - 1),
                )
            ob = outp.tile([P, 512], F32, tag="ob", name="ob")
            if on_scalar:
                nc.scalar.copy(out=ob, in_=ps)
            else:
                nc.vector.tensor_copy(out=ob, in_=ps)
            nc.sync.dma_start(
                out=out.rearrange("(o p) e -> p o e", p=P)[
                    :, t, eo * 512:(eo + 1) * 512
                ],
                in_=ob,
            )

        # Software pipeline: attention head-pairs of chunk qc are interleaved
        # with (via the filler queue) the normalization + O projection of
        # chunk qc-1 and the Q/K/V projections for chunk qc+1.
        emit_kproj(0)
        emit_qproj(0, 0)
        for t in range(4):
            emit_vproj(t)
        for g in range(1, G):
            emit_qproj(g, 0)

        for qc in range(T4):
            if qc + 1 < T4:
                fill_q.append(lambda t4=qc + 1: emit_kproj(t4))
                for g in range(G):
                    fill_q.append(lambda g=g, t4=qc + 1: emit_qproj(g, t4))
            if qc >= 1:
                fill_q.append(lambda q=qc - 1: emit_norm(q))
                for t in range(4 * (qc - 1), 4 * qc):
                    for eo in range(E // 512):
                        fill_q.append(lambda t=t, eo=eo: emit_oproj_eo(t, eo))
            if qc + 1 < T4:
                for t in range(4 * (qc + 1), 4 * (qc + 1) + 4):
                    fill_q.append(lambda t=t: emit_vproj(t))
            for g in range(G):
                emit_attention_pair(
                    qc, g, copies_on_scalar=(qc == T4 - 1 and g == G - 1)
                )
                if fill_q and g < G - 1:
                    fill_q.popleft()()
            # drain: qc+1's projections must land before its attention starts
            while fill_q:
                fill_q.popleft()()

        emit_norm(T4 - 1)
        for t in range(4 * (T4 - 1), 4 * T4):
            for eo in range(E // 512):
                emit_oproj_eo(t, eo, on_scalar=(eo % 2 == 1))

    nc.finalize()
    return nc


def _get_program():
    if "nc" not in _CACHE:
        _CACHE["nc"] = _build_program()
    return _CACHE["nc"]


def _prep_inputs(x, Wq, Wkv, Wo):
    bf = ml_dtypes.bfloat16
    x = np.asarray(x, dtype=np.float32)
    Wq = np.asarray(Wq, dtype=np.float32)
    Wkv = np.asarray(Wkv, dtype=np.float32)
    Wo = np.asarray(Wo, dtype=np.float32)

    # packed local channel order: chunk g holds [head g | head g+4]
    perm = []
    for g in range(G):
        perm.extend(range(g * HD, (g + 1) * HD))
        perm.extend(range((g + 4) * HD, (g + 5) * HD))
    perm = np.asarray(perm)

    tri = np.triu(np.ones((P, P), dtype=np.float32)).astype(bf)  # [k,q]=1 iff q>=k

    # broadcast selection matrices: row i=(4*half+g) -> partitions of half
    selm = np.zeros((8, G, P), dtype=np.float32)
    for g in range(G):
        selm[g, g, 0:HD] = 1.0
        selm[4 + g, g, HD:P] = 1.0
    selm = selm.reshape(8, G * P).astype(bf)

    xTb = [np.ascontiguousarray(x[b].T).astype(bf) for b in range(NB)]
    wq_r, wkv_r, wo_r = [], [], []
    for r in range(NR):
        wq_loc = Wq[r * QL:(r + 1) * QL][perm]            # [512, E] packed
        wq_r.append(np.ascontiguousarray(wq_loc.T).astype(bf))
        k_rows = Wkv[r * KVL:(r + 1) * KVL]               # [128, E]
        v_rows = Wkv[HKV * HD + r * KVL:HKV * HD + (r + 1) * KVL]
        wkv_r.append(np.ascontiguousarray(np.concatenate([k_rows, v_rows], 0).T).astype(bf))
        wo_loc = Wo[:, r * QL:(r + 1) * QL][:, perm]      # [E, 512] packed cols
        wo_r.append(np.ascontiguousarray(wo_loc.T).astype(bf))

    in_maps = []
    for b in range(NB):
        for r in range(NR):
            in_maps.append({
                "xT": xTb[b],
                "wqT": wq_r[r],
                "wkvT": wkv_r[r],
                "woT": wo_r[r],
                "tri": tri,
                "sel": selm,
            })
    return in_maps


def _run(x, Wq, Wkv, Wo, trace=False):
    nc = _get_program()
    in_maps = _prep_inputs(x, Wq, Wkv, Wo)
    res = run_bass_kernel_spmd(nc, in_maps, core_ids=list(range(8)), trace=trace)
    outs = [np.asarray(r["out"], dtype=np.float32) for r in res.results]
    full = np.stack([
        outs[0] + outs[1] + outs[2] + outs[3],
        outs[4] + outs[5] + outs[6] + outs[7],
    ]).astype(np.float32)
    return full, res


def kernel(x, Wq, Wkv, Wo):
    full, _ = _run(x, Wq, Wkv, Wo, trace=False)
    return full


# revision 27
# speedup vs baseline: 1.0062x; 1.0062x over previous
"""GQA attention kernel for Trainium2, 8 NeuronCores.

Problem: B=2, T=2048, E=2048, 32 query heads, 8 KV heads, head_dim=64, causal.
Sharding: 2 (batch) x 4 (tensor-parallel) grid. Each TP rank owns 2 KV heads
(=> 8 query heads, 512 q-channels). Wq/Wkv column-sharded, Wo row-sharded;
per-rank partial outputs are summed on host (cheap vs. on-device collective).

Device kernel per core (all matmuls bf16, fp32 PSUM accumulation):
  1. QT  = Wq_loc  @ x^T   [512, T]   (q channels on partitions, packed so that
                                       q-head i sits at partition half i//4 --
                                       matching its KV head's partition half)
  2. KT  = Wk_loc  @ x^T   [128, T]
  3. V   = x @ Wv_loc^T    [T, 128]   (natural layout, keys on partitions)
  4. flash-style causal attention without max-subtraction (scores ~ N(0,1)):
       S^T[k,q] = K^T.T @ Q^T, P^T = exp(S^T/8), accumulated
       out^T[hd,q] = sum_k (V_aug.T @ P^T) with a ones column giving row sums.
     Normalization is deferred: unnormalized out^T and the row sums are
     staged to SBUF; per 512-query chunk one batched (approx) reciprocal + a
     selection-matrix matmul broadcasts 1/sum across partitions, then a
     single tensor_mul normalizes all 128 channel rows at once.
  5. out_partial = att^T.T @ Wo_loc^T  [T, E] fp32

Scheduling (engines are in-order; deps compile to per-queue counters, so a
dep on one engine serializes behind everything queued earlier there):
  - Heads run as (half0, half1) pairs; both halves' score blocks share one
    2-bank psum tile so a single ACT instruction exps both at once.
  - S matmuls are staggered one k-block ahead of the AV matmuls so the PE
    never waits on the activation (keeps the DVFS p-state at 2.4 GHz; with
    idle gaps the PE drops to 1.2 GHz and every matmul doubles).
  - A filler queue of ACT-independent psum groups (next chunk's Q/K/V
    projections, previous chunk's normalization + O projection) feeds the
    PE at pair start/tail and between pairs, replacing the serial
    phase-by-phase structure with one continuous PE stream.
  - AV matmuls only cover live (unmasked) columns; strictly-masked column
    blocks are never touched, so no memset is needed.
"""

import numpy as np
import ml_dtypes

import concourse.bass as bass
import concourse.mybir as mybir
import concourse.tile as tile
from concourse import bacc
from concourse.bass_utils import run_bass_kernel_spmd

E = 2048
T = 2048
HQ = 32
HKV = 8
HD = 64
G = 4            # query heads per kv head
P = 128
QL = 512         # local q channels per rank (8 heads)
KVL = 128        # local k (or v) channels per rank (2 heads)
NB = 2           # batches
NR = 4           # tensor-parallel ranks
SCALE = 1.0 / 8.0

BF16 = mybir.dt.bfloat16
F32 = mybir.dt.float32

_CACHE = {}


def _build_program():
    from contextlib import ExitStack

    nc = bacc.Bacc(None, target_bir_lowering=False, debug=False)
    xT = nc.declare_dram_parameter("xT", [E, T], BF16, isOutput=False)
    wqT = nc.declare_dram_parameter("wqT", [E, QL], BF16, isOutput=False)
    wkvT = nc.declare_dram_parameter("wkvT", [E, 2 * KVL], BF16, isOutput=False)
    woT = nc.declare_dram_parameter("woT", [QL, E], BF16, isOutput=False)
    tri = nc.declare_dram_parameter("tri", [P, P], BF16, isOutput=False)
    sel = nc.declare_dram_parameter("sel", [8, G * P], BF16, isOutput=False)
    out = nc.declare_dram_parameter("out", [T, E], F32, isOutput=True)

    EC = E // P      # 16 contraction chunks
    TC = T // P      # 16 t chunks of 128
    T4 = T // 512    # 4 t chunks of 512

    with tile.TileContext(nc) as tc, ExitStack() as ctx:
        const = ctx.enter_context(tc.tile_pool(name="const", bufs=1))
        work = ctx.enter_context(tc.tile_pool(name="work", bufs=4))
        tpool = ctx.enter_context(tc.tile_pool(name="tpool", bufs=2))
        outp = ctx.enter_context(tc.tile_pool(name="outp", bufs=4))
        mm = ctx.enter_context(tc.tile_pool(name="mm", bufs=2, space="PSUM"))
        flp = ctx.enter_context(tc.tile_pool(name="flp", bufs=2, space="PSUM"))
        otp = ctx.enter_context(tc.tile_pool(name="otp", bufs=2, space="PSUM"))

        # ---- persistent SBUF tensors ----
        xts = const.tile([P, EC, T], BF16, tag="xts")          # x^T
        wq_s = const.tile([P, EC, QL], BF16, tag="wq")         # Wq^T (packed col order)
        wkv_s = const.tile([P, EC, 2 * KVL], BF16, tag="wkv")  # [K | V] cols
        wo_s = const.tile([P, QL // P, E], BF16, tag="wo")     # Wo^T (packed row order)
        tri_s = const.tile([P, P], BF16, tag="tri")            # causal mask: tri[k,q]=1 iff q>=k
        sel_s = const.tile([8, G, P], BF16, tag="sel")         # broadcast selectors
        qt_s = const.tile([P, G, T], BF16, tag="qt")           # Q^T
        kt_s = const.tile([P, T], BF16, tag="kt")              # K^T
        vag_s = const.tile([P, TC, 2, 66], BF16, tag="vag")    # V_aug per (tchunk, kvhead)
        at_s = const.tile([P, G, T], BF16, tag="at")           # att out^T (normalized in place)
        stage_s = const.tile([P, 8, 512], F32, tag="stage")    # sums staging on partition 64
        sums_s = const.tile([8, T4, 512], F32, tag="sums")     # row sums per (i, qc)
        rec_s = const.tile([8, T4, 512], F32, tag="rec")
        recb_s = const.tile([8, T4, 512], BF16, tag="recb")

        # x^T is loaded per 512-t chunk so the first projections start early;
        # the weights needed by the first psum groups come first
        xTr = xT.rearrange("(o p) t -> p o t", p=P)
        nc.sync.dma_start(out=xts[:, :, 0:512], in_=xTr[:, :, 0:512])
        nc.sync.dma_start(out=wkv_s, in_=wkvT.rearrange("(o p) c -> p o c", p=P))
        nc.sync.dma_start(out=wq_s, in_=wqT.rearrange("(o p) q -> p o q", p=P))
        for t4 in range(1, T4):
            nc.sync.dma_start(
                out=xts[:, :, t4 * 512:(t4 + 1) * 512],
                in_=xTr[:, :, t4 * 512:(t4 + 1) * 512],
            )
        nc.sync.dma_start(out=wo_s, in_=woT.rearrange("(o p) e -> p o e", p=P))
        nc.sync.dma_start(out=tri_s, in_=tri[:])
        nc.sync.dma_start(out=sel_s, in_=sel.rearrange("p (b m) -> p b m", b=G))
        nc.vector.memset(vag_s[:, :, :, 64:66], 1.0)  # ones col (65) + pad (66)

        # ---- projection psum groups (emitted directly or as filler thunks) --
        def emit_qproj(g, t4):
            ps = flp.tile([P, 512], F32, tag="fl", name="ps")
            for e in range(EC):
                nc.tensor.matmul(
                    ps,
                    lhsT=wq_s[:, e, g * P:(g + 1) * P],
                    rhs=xts[:, e, t4 * 512:(t4 + 1) * 512],
                    start=(e == 0),
                    stop=(e == EC - 1),
                )
            nc.vector.tensor_copy(out=qt_s[:, g, t4 * 512:(t4 + 1) * 512], in_=ps)

        def emit_kproj(t4):
            ps = flp.tile([P, 512], F32, tag="fl", name="ps")
            for e in range(EC):
                nc.tensor.matmul(
                    ps,
                    lhsT=wkv_s[:, e, 0:KVL],
                    rhs=xts[:, e, t4 * 512:(t4 + 1) * 512],
                    start=(e == 0),
                    stop=(e == EC - 1),
                )
            nc.vector.tensor_copy(out=kt_s[:, t4 * 512:(t4 + 1) * 512], in_=ps)

        def emit_vproj(t):
            ps = flp.tile([P, 512], F32, tag="fl", name="ps")
            psv = ps[:, :KVL]
            for e in range(EC):
                nc.tensor.matmul(
                    psv,
                    lhsT=xts[:, e, t * P:(t + 1) * P],
                    rhs=wkv_s[:, e, KVL:2 * KVL],
                    start=(e == 0),
                    stop=(e == EC - 1),
                )
            for h in range(2):
                nc.vector.tensor_copy(
                    out=vag_s[:, t, h, 0:HD], in_=psv[:, h * HD:(h + 1) * HD]
                )

        # ---- attention + per-chunk normalization + O projection ----
        # local q-head i (0..7): kv half = i//4, qt chunk = i%4, partition base = (i//4)*64
        # Heads are processed in (half0, half1) pairs so one head's exp (ACT)
        # overlaps the other's matmuls (PE); normalization + O projection of
        # chunk qc is emitted after the attention of chunk qc+1 (software
        # pipelining) so the reciprocal never stalls the PE queue.
        from collections import deque

        fill_q = deque()   # small PE work items with no ACT dependency

        def emit_attention_pair(qc, g, copies_on_scalar=False):
            # head pair (i=g, i=g+4): both halves' scores go into one 2-bank
            # psum tile so a single ACT instruction exps both; S matmuls run
            # two k-blocks ahead of the AV matmuls so the PE never waits on
            # the activation (stays at full clock). When the stagger runs out
            # of S matmuls (pair tail), filler thunks keep the PE stream fed.
            q_sl = slice(qc * 512, (qc + 1) * 512)
            kmax = 4 * qc + 3
            ots = [otp.tile([65, 512], F32, tag="ot", name=f"ot{h}") for h in range(2)]
            st_tiles = {}

            def c0_of(kb):
                return max(kb - 4 * qc, 0) * P

            def emit_s(kb):
                c0 = c0_of(kb)
                st2 = mm.tile([P, 2, 512], F32, tag="st", name="st2")
                st_tiles[kb] = st2
                for half in range(2):
                    pb = half * HD
                    nc.tensor.matmul(
                        st2[:, half, c0:512],
                        lhsT=kt_s[pb:pb + HD, kb * P:(kb + 1) * P],
                        rhs=qt_s[pb:pb + HD, g, qc * 512 + c0:(qc + 1) * 512],
                        start=True,
                        stop=True,
                    )

            emit_s(0)
            if fill_q:
                fill_q.popleft()()   # covers exp(0) latency at pair start
            for kb in range(kmax + 1):
                if kb + 1 <= kmax:
                    emit_s(kb + 1)
                elif fill_q:
                    fill_q.popleft()()
                j = kb - 4 * qc
                c0 = c0_of(kb)
                st2 = st_tiles.pop(kb)
                pt2 = work.tile([P, 2, 512], BF16, tag="pt", name="pt2")
                nc.scalar.activation(
                    out=pt2[:, :, c0:512],
                    in_=st2[:, :, c0:512],
                    func=mybir.ActivationFunctionType.Exp,
                    scale=SCALE,
                )
                for half in range(2):
                    if j >= 0:
                        nc.vector.tensor_mul(
                            out=pt2[:, half, c0:c0 + P],
                            in0=pt2[:, half, c0:c0 + P],
                            in1=tri_s,
                        )
                    nc.tensor.matmul(
                        ots[half][:, c0:512],
                        lhsT=vag_s[:, kb, half, 0:65],
                        rhs=pt2[:, half, c0:512],
                        start=(kb == 0),
                        stop=(kb == kmax),
                        skip_group_check=True,
                    )

            # stage unnormalized output + row sums; normalize later in bulk
            ceng = nc.scalar.copy if copies_on_scalar else nc.vector.tensor_copy
            for half in range(2):
                ot = ots[half]
                i = 4 * half + g
                ceng(out=stage_s[64:65, i, :], in_=ot[64:65, :])
                if half == 0:
                    ceng(out=at_s[0:HD, g, q_sl], in_=ot[0:HD, :])
                else:
                    # DVE lanes can't cross partitions; write at base 0 then
                    # DMA-shift SBUF->SBUF into partitions 64..127
                    tmp = tpool.tile([HD, 512], BF16, tag="tmp")
                    ceng(out=tmp, in_=ot[0:HD, :])
                    nc.sync.dma_start(out=at_s[HD:P, g, q_sl], in_=tmp)
            for i in (g, 4 + g):
                nc.sync.dma_start(
                    out=sums_s[i:i + 1, qc, :], in_=stage_s[64:65, i, :]
                )

        def emit_norm(qc):
            q_sl = slice(qc * 512, (qc + 1) * 512)
            # batched normalization for this q chunk: 8 rows at once
            nc.vector.reciprocal_approx_fast(out=rec_s[0:8, qc, :], in_=sums_s[0:8, qc, :])
            nc.scalar.copy(out=recb_s[0:8, qc, :], in_=rec_s[0:8, qc, :])
            for g in range(G):
                bc = flp.tile([P, 512], F32, tag="fl", name="bc")
                nc.tensor.matmul(
                    bc,
                    lhsT=sel_s[:, g, :],
                    rhs=recb_s[0:8, qc, :],
                    start=True,
                    stop=True,
                )
                nc.vector.tensor_mul(
                    out=at_s[:, g, q_sl], in0=at_s[:, g, q_sl], in1=bc
                )

        def emit_oproj_eo(t, eo, on_scalar=False):
            # ---- O projection, one [128 t, 512 eo] psum group ----
            ps = flp.tile([P, 512], F32, tag="fl", name="ps")
            for cc in range(QL // P):
                nc.tensor.matmul(
                    ps,
                    lhsT=at_s[:, cc, t * P:(t + 1) * P],
                    rhs=wo_s[:, cc, eo * 512:(eo + 1) * 512],
                    start=(cc == 0),
                    stop=(cc == QL // P - 1),
                )
            ob = outp.tile([P, 512], F32, tag="ob", name="ob")
            if on_scalar:
                nc.scalar.copy(out=ob, in_=ps)
            else:
                nc.vector.tensor_copy(out=ob, in_=ps)
            nc.sync.dma_start(
                out=out.rearrange("(o p) e -> p o e", p=P)[
                    :, t, eo * 512:(eo + 1) * 512
                ],
                in_=ob,
            )

        # Software pipeline: attention head-pairs of chunk qc are interleaved
        # with (via the filler queue) the normalization + O projection of
        # chunk qc-1 and the Q/K/V projections for chunk qc+1.
        emit_kproj(0)
        emit_qproj(0, 0)
        for t in range(4):
            emit_vproj(t)
        for g in range(1, G):
            emit_qproj(g, 0)

        for qc in range(T4):
            if qc + 1 < T4:
                fill_q.append(lambda t4=qc + 1: emit_kproj(t4))
                for g in range(G):
                    fill_q.append(lambda g=g, t4=qc + 1: emit_qproj(g, t4))
            if qc >= 1:
                fill_q.append(lambda q=qc - 1: emit_norm(q))
                for t in range(4 * (qc - 1), 4 * qc):
                    for eo in range(E // 512):
                        fill_q.append(lambda t=t, eo=eo: emit_oproj_eo(t, eo))
            if qc + 1 < T4:
                for t in range(4 * (qc + 1), 4 * (qc + 1) + 4):
                    fill_q.append(lambda t=t: emit_vproj(t))
            for g in range(G):
                emit_attention_pair(
                    qc, g, copies_on_scalar=(qc == T4 - 1 and g == G - 1)
                )
                if fill_q and g < G - 1:
                    fill_q.popleft()()
            # drain: qc+1's projections must land before its attention starts
            while fill_q:
                fill_q.popleft()()

        emit_norm(T4 - 1)
        for t in range(4 * (T4 - 1), 4 * T4):
            for eo in range(E // 512):
                emit_oproj_eo(t, eo, on_scalar=(eo % 2 == 1))

    nc.finalize()
    return nc


def _get_program():
    if "nc" not in _CACHE:
        _CACHE["nc"] = _build_program()
    return _CACHE["nc"]


def _prep_inputs(x, Wq, Wkv, Wo):
    bf = ml_dtypes.bfloat16
    x = np.asarray(x, dtype=np.float32)
    Wq = np.asarray(Wq, dtype=np.float32)
    Wkv = np.asarray(Wkv, dtype=np.float32)
    Wo = np.asarray(Wo, dtype=np.float32)

    # packed local channel order: chunk g holds [head g | head g+4]
    perm = []
    for g in range(G):
        perm.extend(range(g * HD, (g + 1) * HD))
        perm.extend(range((g + 4) * HD, (g + 5) * HD))
    perm = np.asarray(perm)

    tri = np.triu(np.ones((P, P), dtype=np.float32)).astype(bf)  # [k,q]=1 iff q>=k

    # broadcast selection matrices: row i=(4*half+g) -> partitions of half
    selm = np.zeros((8, G, P), dtype=np.float32)
    for g in range(G):
        selm[g, g, 0:HD] = 1.0
        selm[4 + g, g, HD:P] = 1.0
    selm = selm.reshape(8, G * P).astype(bf)

    xTb = [np.ascontiguousarray(x[b].T).astype(bf) for b in range(NB)]
    wq_r, wkv_r, wo_r = [], [], []
    for r in range(NR):
        wq_loc = Wq[r * QL:(r + 1) * QL][perm]            # [512, E] packed
        wq_r.append(np.ascontiguousarray(wq_loc.T).astype(bf))
        k_rows = Wkv[r * KVL:(r + 1) * KVL]               # [128, E]
        v_rows = Wkv[HKV * HD + r * KVL:HKV * HD + (r + 1) * KVL]
        wkv_r.append(np.ascontiguousarray(np.concatenate([k_rows, v_rows], 0).T).astype(bf))
        wo_loc = Wo[:, r * QL:(r + 1) * QL][:, perm]      # [E, 512] packed cols
        wo_r.append(np.ascontiguousarray(wo_loc.T).astype(bf))

    in_maps = []
    for b in range(NB):
        for r in range(NR):
            in_maps.append({
                "xT": xTb[b],
                "wqT": wq_r[r],
                "wkvT": wkv_r[r],
                "woT": wo_r[r],
                "tri": tri,
                "sel": selm,
            })
    return in_maps


def _run(x, Wq, Wkv, Wo, trace=False):
    nc = _get_program()
    in_maps = _prep_inputs(x, Wq, Wkv, Wo)
    res = run_bass_kernel_spmd(nc, in_maps, core_ids=list(range(8)), trace=trace)
    outs = [np.asarray(r["out"], dtype=np.float32) for r in res.results]
    full = np.stack([
        outs[0] + outs[1] + outs[2] + outs[3],
        outs[4] + outs[5] + outs[6] + outs[7],
    ]).astype(np.float32)
    return full, res


def kernel(x, Wq, Wkv, Wo):
    full, _ = _run(x, Wq, Wkv, Wo, trace=False)
    return full
